# revision 9
# baseline (speedup 1.0000x reference)
"""Trainium2 Bass kernel for a 2-layer GAT (B=8, N=1024, F=256, D=64, H=8, C=256).

Sharding: data-parallel over batch — one batch element per NeuronCore (8 cores).

Layer-1 attention uses a host-fitted rank-2 separable factorization of the
scalar kernel g(s) = exp(LeakyReLU(s)) evaluated at s = sl_i + sr_j:

    g(sl_i + sr_j) ~= phi0(sl_i) psi0(sr_j) + phi1(sl_i) psi1(sr_j)

(per batch, per head, SVD of g on the realized [sl]x[sr] box). The masked
softmax aggregation then needs NO N^2 elementwise work:

    num_i = phi0_i (M @ (psi0 . h))_i + phi1_i (M @ (psi1 . h))_i
    Z_i   = phi0_i (M @ psi0)_i      + phi1_i (M @ psi1)_i
    attn-out_i = num_i / Z_i                     (phi0 cancels; rho=phi1/phi0)

so layer-1 is mask matmuls (lhsT = adjT chunk, shared across heads/ranks)
over value blocks psi_k.h. psi/rho ship as tiny per-node columns and are
broadcast-expanded on device (keeps startup DMA small). Layer 2 (single
head, C=256) keeps the exact masked-exp sweep; tl/tr are produced early via
2-col matmuls so the g-projection can overlap the serial sweep.
"""

import numpy as np
import ml_dtypes
from contextlib import ExitStack

BF16 = ml_dtypes.bfloat16
B, N, F, D, H, C = 8, 1024, 256, 64, 8, 256
HD = H * D  # 512
RK = 2  # separable rank for layer-1 attention
ALPHA = 0.2

_CACHE = {}


def _build_program():
    import concourse.bacc as bacc
    import concourse.bass as bass
    import concourse.mybir as mybir
    from concourse.tile import TileContext
    from concourse.masks import make_identity

    dt = mybir.dt
    Alu = mybir.AluOpType
    Act = mybir.ActivationFunctionType

    nc = bacc.Bacc()

    xt = nc.declare_dram_parameter("xt", [F + 1, N], dt.bfloat16, isOutput=False)
    xs = nc.declare_dram_parameter("xs", [N, F], dt.float32, isOutput=False)
    msk = nc.declare_dram_parameter("msk", [N, N], dt.bfloat16, isOutput=False)
    wp = nc.declare_dram_parameter("wp", [F + 1, HD], dt.bfloat16, isOutput=False)
    psicol = nc.declare_dram_parameter(
        "psicol", [N, RK * H], dt.bfloat16, isOutput=False
    )
    rhof = nc.declare_dram_parameter("rhof", [N, H], dt.float32, isOutput=False)
    wo = nc.declare_dram_parameter("wo", [HD + 1, C + 2], dt.bfloat16, isOutput=False)
    out_d = nc.declare_dram_parameter("out", [N, C], dt.float32, isOutput=True)

    rows_d = nc.dram_tensor("rows_bounce", [2, N], dt.bfloat16)

    NCH = N // 128  # 8 chunks of 128 nodes

    def bcast128(row_ap):
        # [1, N] DRAM row -> [128, N] partition-broadcast read for DMA
        return bass.AP(
            tensor=row_ap.tensor,
            offset=row_ap.offset,
            ap=[[0, 128]] + list(row_ap.ap),
        )

    with TileContext(nc) as tc:
        with ExitStack() as ctx:
            cons = ctx.enter_context(tc.tile_pool(name="cons", bufs=1))
            bc = ctx.enter_context(tc.tile_pool(name="bc", bufs=2))
            eb = ctx.enter_context(tc.tile_pool(name="eb", bufs=1))
            tb = ctx.enter_context(tc.tile_pool(name="tb", bufs=1))
            wk = ctx.enter_context(tc.tile_pool(name="wk", bufs=3))
            sm = ctx.enter_context(tc.tile_pool(name="sm", bufs=3))
            pa0p = ctx.enter_context(tc.tile_pool(name="pa0", bufs=2, space="PSUM"))
            pa1p = ctx.enter_context(tc.tile_pool(name="pa1", bufs=2, space="PSUM"))
            pzp = ctx.enter_context(tc.tile_pool(name="pzp", bufs=1, space="PSUM"))
            pm2 = ctx.enter_context(tc.tile_pool(name="pm2", bufs=2, space="PSUM"))
            ptp = ctx.enter_context(tc.tile_pool(name="ptp", bufs=1, space="PSUM"))

            # ---------- constants ----------
            ident_f = cons.tile([128, 128], dt.float32)
            make_identity(nc, ident_f[:, :])
            ident_b = cons.tile([128, 128], dt.bfloat16)
            make_identity(nc, ident_b[:, :])

            # ---------- DMAs: critical path first (sync q), bulk on gpsimd q
            xt_sb = cons.tile([128, 2 * N], dt.bfloat16)
            nc.sync.dma_start(out=xt_sb[:, 0:N], in_=xt[0:128, :])
            nc.sync.dma_start(out=xt_sb[:, N : 2 * N], in_=xt[128:256, :])
            xt_one = cons.tile([1, N], dt.bfloat16)
            nc.sync.dma_start(out=xt_one[:, :], in_=xt[256:257, :])

            wp_sb = cons.tile([128, 2 * HD], dt.bfloat16)
            nc.sync.dma_start(out=wp_sb[:, 0:HD], in_=wp[0:128, :])
            nc.sync.dma_start(out=wp_sb[:, HD : 2 * HD], in_=wp[128:256, :])
            wp_one = cons.tile([1, HD], dt.bfloat16)
            nc.sync.dma_start(out=wp_one[:, :], in_=wp[256:257, :])

            psicol_sb = cons.tile([128, NCH * RK * H], dt.bfloat16)
            nc.sync.dma_start(
                out=psicol_sb[:, :].rearrange("p (n q) -> p n q", q=RK * H),
                in_=psicol[:, :].rearrange("(n p) q -> p n q", p=128),
            )
            rhof_sb = cons.tile([128, NCH * H], dt.float32)
            nc.sync.dma_start(
                out=rhof_sb[:, :].rearrange("p (n q) -> p n q", q=H),
                in_=rhof[:, :].rearrange("(n p) q -> p n q", p=128),
            )

            msk_sb = cons.tile([128, NCH * N], dt.bfloat16)
            for c in range(NCH):
                nc.gpsimd.dma_start(
                    out=msk_sb[:, c * N : (c + 1) * N],
                    in_=msk[c * 128 : (c + 1) * 128, :],
                )

            wo_sb = cons.tile([128, 4 * (C + 2)], dt.bfloat16)
            for k in range(4):
                nc.gpsimd.dma_start(
                    out=wo_sb[:, k * (C + 2) : (k + 1) * (C + 2)],
                    in_=wo[k * 128 : (k + 1) * 128, :],
                )
            wo_one = cons.tile([1, C + 2], dt.bfloat16)
            nc.gpsimd.dma_start(out=wo_one[:, :], in_=wo[HD : HD + 1, :])

            # ---------- PE p-state warmup during the DMA window ----------
            pwu = pm2.tile([128, 128], dt.float32, tag="mm2")
            for _ in range(36):
                nc.tensor.matmul(
                    pwu[:, :], ident_b[:, :], ident_b[:, :], start=True, stop=True
                )

            # ---------- expand psi columns to 64-wide head blocks ----------
            psirep_sb = cons.tile([128, NCH * RK * HD], dt.bfloat16)
            for n in range(NCH):
                for k in range(RK):
                    nc.vector.tensor_copy(
                        out=psirep_sb[
                            :, n * RK * HD + k * HD : n * RK * HD + (k + 1) * HD
                        ].rearrange("p (h s) -> p h s", s=D),
                        in_=psicol_sb[
                            :, n * RK * H + k * H : n * RK * H + (k + 1) * H
                        ]
                        .rearrange("p (h s) -> p h s", s=1)
                        .to_broadcast([128, H, D]),
                    )

            # ---------- phase 1: h = x@W_all ; V = psi_k . h ----------
            hx = cons.tile([128, NCH * HD], dt.bfloat16)
            v_sb = cons.tile([128, NCH * RK * HD], dt.bfloat16)
            for n in range(NCH):
                hp_pool = pa0p if n % 2 == 0 else pa1p
                ph = hp_pool.tile(
                    [128, HD], dt.float32, tag="a0" if n % 2 == 0 else "a1"
                )
                for k in range(2):
                    lt = xt_sb[:, k * N + n * 128 : k * N + n * 128 + 128]
                    nc.tensor.matmul(
                        ph[:, :], lt, wp_sb[:, k * HD : (k + 1) * HD],
                        start=(k == 0), stop=False,
                    )
                lt1 = xt_one[:, n * 128 : n * 128 + 128]
                nc.tensor.matmul(ph[:, :], lt1, wp_one[:, :], start=False, stop=True)
                nc.scalar.activation(
                    hx[:, n * HD : (n + 1) * HD], ph[:, :], Act.Copy
                )
                for k in range(RK):
                    base = n * RK * HD + k * HD
                    nc.vector.tensor_tensor(
                        out=v_sb[:, base : base + HD],
                        in0=hx[:, n * HD : (n + 1) * HD],
                        in1=psirep_sb[:, base : base + HD],
                        op=Alu.mult,
                    )

            # ---------- phase 2: L1 agg -> z -> zT -> tl/tr ----------
            z_sb = cons.tile([128, NCH * HD], dt.bfloat16)
            zt_sb = cons.tile([128, 4 * N], dt.bfloat16)
            zt_one = cons.tile([1, N], dt.bfloat16)
            nc.vector.memset(zt_one[:, :], 1.0)
            gx = cons.tile([128, NCH * 260], dt.bfloat16)
            nc.vector.memset(
                gx[:, :].rearrange("p (n s) -> p n s", s=260)[:, :, 256:257], 1.0
            )
            glgr = cons.tile([128, NCH * 2], dt.float32)

            pz_all = pzp.tile([128, 2 * (RK * H + 2)], dt.float32, tag="az")
            for ic in range(NCH):
                pa0 = pa0p.tile([128, HD], dt.float32, tag="a0")
                pa1 = pa1p.tile([128, HD], dt.float32, tag="a1")
                pzo = (ic % 2) * (RK * H + 2)
                pz = pz_all[:, pzo : pzo + RK * H + 2]
                for jc in range(NCH):
                    w = msk_sb[:, jc * N + ic * 128 : jc * N + ic * 128 + 128]
                    st = jc == 0
                    sp = jc == NCH - 1
                    nc.tensor.matmul(
                        pa0[:, :], w, v_sb[:, jc * RK * HD : jc * RK * HD + HD],
                        start=st, stop=sp,
                    )
                    nc.tensor.matmul(
                        pa1[:, :], w,
                        v_sb[:, jc * RK * HD + HD : jc * RK * HD + 2 * HD],
                        start=st, stop=sp,
                    )
                    nc.tensor.matmul(
                        pz[:, 0 : RK * H], w,
                        psicol_sb[:, jc * RK * H : (jc + 1) * RK * H],
                        start=st, stop=sp,
                    )
                # exits to bf16
                n0 = wk.tile([128, HD], dt.bfloat16, tag="n0")
                n1 = wk.tile([128, HD], dt.bfloat16, tag="n1")
                nc.scalar.activation(n0[:, :], pa0[:, :], Act.Copy)
                nc.scalar.activation(n1[:, :], pa1[:, :], Act.Copy)
                # Z = pz[:,0:8] + rho . pz[:,8:16]  (fp32)
                zt1 = wk.tile([128, 2 * H], dt.float32, tag="zt1")
                nc.vector.tensor_tensor(
                    out=zt1[:, 0:H], in0=pz[:, H : 2 * H],
                    in1=rhof_sb[:, ic * H : (ic + 1) * H], op=Alu.mult,
                )
                nc.vector.tensor_tensor(
                    out=zt1[:, H : 2 * H], in0=zt1[:, 0:H], in1=pz[:, 0:H],
                    op=Alu.add,
                )
                rz = wk.tile([128, H], dt.float32, tag="rz")
                nc.vector.reciprocal(
                    rz[:, :].rearrange("p (h s) -> p h s", s=1),
                    zt1[:, H : 2 * H].rearrange("p (h s) -> p h s", s=1),
                )
                # num = n0 + rhorep . n1 ; hh = num . rzrep
                rhorep = wk.tile([128, HD], dt.bfloat16, tag="rhorep")
                nc.vector.tensor_copy(
                    out=rhorep[:, :].rearrange("p (h s) -> p h s", s=D),
                    in_=rhof_sb[:, ic * H : (ic + 1) * H]
                    .rearrange("p (h s) -> p h s", s=1)
                    .to_broadcast([128, H, D]),
                )
                num = wk.tile([128, HD], dt.bfloat16, tag="num")
                nc.vector.tensor_tensor(
                    out=num[:, :], in0=n1[:, :], in1=rhorep[:, :], op=Alu.mult
                )
                nc.vector.tensor_tensor(
                    out=num[:, :], in0=num[:, :], in1=n0[:, :], op=Alu.add
                )
                rzrep = wk.tile([128, HD], dt.bfloat16, tag="rzrep")
                nc.vector.tensor_copy(
                    out=rzrep[:, :].rearrange("p (h s) -> p h s", s=D),
                    in_=rz[:, :]
                    .rearrange("p (h s) -> p h s", s=1)
                    .to_broadcast([128, H, D]),
                )
                hh = wk.tile([128, HD], dt.bfloat16, tag="hh")
                nc.vector.tensor_tensor(
                    out=hh[:, :], in0=num[:, :], in1=rzrep[:, :], op=Alu.mult
                )
                # ELU: z = relu(hh) + min(exp(hh)-1, 0)
                ee = wk.tile([128, HD], dt.bfloat16, tag="ee")
                nc.scalar.activation(ee[:, :], hh[:, :], Act.Exp)
                r1 = wk.tile([128, HD], dt.bfloat16, tag="r1")
                nc.vector.tensor_scalar(
                    out=r1[:, :], in0=ee[:, :], scalar1=-1.0, scalar2=0.0,
                    op0=Alu.add, op1=Alu.min,
                )
                nc.vector.scalar_tensor_tensor(
                    out=z_sb[:, ic * HD : (ic + 1) * HD],
                    in0=hh[:, :], scalar=0.0, in1=r1[:, :],
                    op0=Alu.max, op1=Alu.add,
                )

                # zT for this node chunk (4 transposed 128x128 blocks)
                pzi = pm2.tile([128, 4 * 128], dt.bfloat16, tag="mm2")
                for kc in range(4):
                    nc.tensor.transpose(
                        pzi[:, kc * 128 : (kc + 1) * 128],
                        z_sb[:, ic * HD + kc * 128 : ic * HD + kc * 128 + 128],
                        ident_b[:, :],
                    )
                nc.vector.tensor_copy(
                    out=zt_sb[:, :]
                    .rearrange("p (kc n) -> p kc n", n=N)[:, :, ic * 128 : ic * 128 + 128],
                    in_=pzi[:, :].rearrange("p (kc s) -> p kc s", s=128),
                )

                # tl/tr for this chunk via 2-col matmuls (u_l/u_r cols of wo)
                ptl = pz[:, RK * H : RK * H + 2]
                for kc in range(4):
                    nc.tensor.matmul(
                        ptl,
                        zt_sb[:, kc * N + ic * 128 : kc * N + ic * 128 + 128],
                        wo_sb[:, kc * (C + 2) + C : kc * (C + 2) + C + 2],
                        start=(kc == 0), stop=False,
                    )
                nc.tensor.matmul(
                    ptl, zt_one[:, ic * 128 : ic * 128 + 128],
                    wo_one[:, C : C + 2], start=False, stop=True,
                )
                nc.vector.tensor_copy(
                    out=glgr[:, ic * 2 : (ic + 1) * 2], in_=ptl
                )
                gb16 = wk.tile([128, 2], dt.bfloat16, tag="gb16")
                nc.vector.tensor_copy(
                    out=gb16[:, :], in_=glgr[:, ic * 2 : (ic + 1) * 2]
                )
                pt2 = ptp.tile([2, 128], dt.bfloat16, tag="tp")
                nc.tensor.transpose(pt2[:, :], gb16[:, :], ident_b[:, :])
                gr2 = wk.tile([2, 128], dt.bfloat16, tag="gr2")
                nc.vector.tensor_copy(out=gr2[:, :], in_=pt2[:, :])
                nc.sync.dma_start(
                    out=rows_d[0:2, ic * 128 : (ic + 1) * 128], in_=gr2[:, :]
                )

            # ---------- phase 3a: L2 sweep prep (tl row broadcast) ----------
            CA = 3  # chunks LeakyReLU'd by ACT (Prelu bias trick); rest DVE
            glb = bc.tile([128, N], dt.bfloat16, tag="slb")
            nc.sync.dma_start(out=glb[:, :], in_=bcast128(rows_d[0:1, :]))
            e2 = eb.tile([128, NCH * N], dt.bfloat16, tag="e")
            for c in range(CA):
                nc.scalar.activation(
                    e2[:, c * N : (c + 1) * N], glb[:, :], Act.Prelu,
                    bias=glgr[:, c * 2 + 1 : c * 2 + 2], scale=1.0, alpha=ALPHA,
                )
            s = CA * N
            w_ = (NCH - CA) * N
            for c in range(CA, NCH):
                nc.vector.tensor_scalar(
                    out=e2[:, c * N : (c + 1) * N], in0=glb[:, :],
                    scalar1=glgr[:, c * 2 + 1 : c * 2 + 2], scalar2=None,
                    op0=Alu.add,
                )
            t = tb.tile([128, (NCH - CA) * N], dt.bfloat16, tag="t")
            nc.vector.tensor_scalar(
                out=t[:, 0:w_], in0=e2[:, s : s + w_], scalar1=ALPHA,
                scalar2=None, op0=Alu.mult,
            )
            nc.vector.tensor_tensor(
                out=e2[:, s : s + w_], in0=t[:, 0:w_], in1=e2[:, s : s + w_],
                op=Alu.max,
            )

            # ---------- phase 3b: g-projection (overlaps the sweep) ----------
            for ic in range(NCH):
                pg = pm2.tile([128, C + 2], dt.float32, tag="mm2")
                for kc in range(4):
                    nc.tensor.matmul(
                        pg[:, :],
                        zt_sb[:, kc * N + ic * 128 : kc * N + ic * 128 + 128],
                        wo_sb[:, kc * (C + 2) : (kc + 1) * (C + 2)],
                        start=(kc == 0), stop=False,
                    )
                nc.tensor.matmul(
                    pg[:, :], zt_one[:, ic * 128 : ic * 128 + 128], wo_one[:, :],
                    start=False, stop=True,
                )
                nc.vector.tensor_copy(
                    out=gx[:, ic * 260 : ic * 260 + C], in_=pg[:, 0:C]
                )

            # exp + mask in quarters for ACT/DVE pipelining
            QN = NCH * N // 4
            for q in range(4):
                nc.scalar.activation(
                    e2[:, q * QN : (q + 1) * QN], e2[:, q * QN : (q + 1) * QN],
                    Act.Exp,
                )
                nc.vector.tensor_tensor(
                    out=e2[:, q * QN : (q + 1) * QN],
                    in0=e2[:, q * QN : (q + 1) * QN],
                    in1=msk_sb[:, q * QN : (q + 1) * QN], op=Alu.mult,
                )

            # ---------- phase 4: L2 aggregation + ELU + residual ----------
            for ic in range(NCH):
                po = pa0p.tile([128, HD], dt.float32, tag="a0")
                for jc in range(NCH):
                    nc.tensor.matmul(
                        po[:, 0 : C + 1],
                        e2[:, jc * N + ic * 128 : jc * N + ic * 128 + 128],
                        gx[:, jc * 260 : jc * 260 + C + 1],
                        start=(jc == 0), stop=(jc == NCH - 1),
                    )
                rz2 = sm.tile([128, 1], dt.float32, tag="rz2")
                nc.vector.reciprocal(rz2[:, :], po[:, C : C + 1])
                y = sm.tile([128, C], dt.bfloat16, tag="y")
                nc.vector.tensor_scalar(
                    out=y[:, :], in0=po[:, 0:C], scalar1=rz2[:, :], scalar2=None,
                    op0=Alu.mult,
                )
                e3 = sm.tile([128, C], dt.bfloat16, tag="e3")
                nc.scalar.activation(e3[:, :], y[:, :], Act.Exp)
                r2 = sm.tile([128, C], dt.bfloat16, tag="r2")
                nc.vector.tensor_scalar(
                    out=r2[:, :], in0=e3[:, :], scalar1=-1.0, scalar2=0.0,
                    op0=Alu.add, op1=Alu.min,
                )
                el = sm.tile([128, C], dt.bfloat16, tag="el")
                nc.vector.scalar_tensor_tensor(
                    out=el[:, :], in0=y[:, :], scalar=0.0, in1=r2[:, :],
                    op0=Alu.max, op1=Alu.add,
                )
                xs5 = sm.tile([128, F], dt.float32, tag="xs5")
                nc.sync.dma_start(
                    out=xs5[:, :], in_=xs[ic * 128 : (ic + 1) * 128, :]
                )
                ofin = sm.tile([128, C], dt.float32, tag="ofin")
                nc.vector.tensor_tensor(
                    out=ofin[:, :], in0=el[:, :], in1=xs5[:, :], op=Alu.add,
                )
                nc.sync.dma_start(
                    out=out_d[ic * 128 : (ic + 1) * 128, :], in_=ofin[:, :]
                )

    nc.compile()
    return nc


def get_program():
    if "nc" not in _CACHE:
        _CACHE["nc"] = _build_program()
    return _CACHE["nc"]


def _fit_rank2(sl, sr, ngrid=257):
    """Fit g(x+y)=exp(LeakyReLU(x+y)) ~= sum_k phi_k(x) psi_k(y), rank RK,
    on the realized box. Returns (rho[N] fp32, psi[N, RK] fp32)."""
    pad_x = 1e-3 * (sl.max() - sl.min()) + 1e-6
    pad_y = 1e-3 * (sr.max() - sr.min()) + 1e-6
    xs = np.linspace(sl.min() - pad_x, sl.max() + pad_x, ngrid)
    ys = np.linspace(sr.min() - pad_y, sr.max() + pad_y, ngrid)
    ss = xs[:, None] + ys[None, :]
    G = np.exp(np.where(ss >= 0, ss, ALPHA * ss))
    U, S, Vt = np.linalg.svd(G, full_matrices=False)
    phi_g = U[:, :RK] * S[:RK]
    psi_g = Vt[:RK].T
    if phi_g[:, 0].mean() < 0:
        phi_g[:, 0] *= -1.0
        psi_g[:, 0] *= -1.0
    phi = np.stack([np.interp(sl, xs, phi_g[:, k]) for k in range(RK)], axis=1)
    psi = np.stack([np.interp(sr, ys, psi_g[:, k]) for k in range(RK)], axis=1)
    assert np.all(phi[:, 0] > 0), "phi0 must be positive"
    rho = phi[:, 1] / phi[:, 0]
    return rho.astype(np.float32), psi.astype(np.float32)


def make_in_maps(x, adj, W, Wb, a, ab, Wo, Wob, ao, aob):
    x = np.asarray(x, np.float32)
    adj = np.asarray(adj)
    W = np.asarray(W, np.float32)
    Wb = np.asarray(Wb, np.float32)
    a = np.asarray(a, np.float32)
    ab = np.asarray(ab, np.float32)
    Wo = np.asarray(Wo, np.float32)
    Wob = np.asarray(Wob, np.float32)
    ao = np.asarray(ao, np.float32)
    aob = np.asarray(aob, np.float32)

    # W_all[f, h*D+d] = W[h, f, d];  Wb row flattened the same way
    W_all = W.transpose(1, 0, 2).reshape(F, HD)
    wb_row = Wb.reshape(1, HD)
    wp = np.concatenate([W_all, wb_row], axis=0).astype(BF16)  # [257, 512]

    # sl/sr per-node linear maps of x, folded on the host (fp32)
    V_l = np.einsum("hfd,hd->fh", W, a[:, :D]).astype(np.float32)
    V_r = np.einsum("hfd,hd->fh", W, a[:, D:]).astype(np.float32)
    const_l = (Wb * a[:, :D]).sum(1) + ab  # [H]
    const_r = (Wb * a[:, D:]).sum(1)
    sl_all = np.einsum("bnf,fh->bhn", x, V_l) + const_l[None, :, None]  # [B,H,N]
    sr_all = np.einsum("bnf,fh->bhn", x, V_r) + const_r[None, :, None]  # [B,H,N]

    u_l = Wo @ ao[:C]  # [512]
    u_r = Wo @ ao[C:]
    wo_top = np.concatenate([Wo, u_l[:, None], u_r[:, None]], axis=1)  # [512, 258]
    wo_bot = np.concatenate(
        [Wob, [Wob @ ao[:C] + aob], [Wob @ ao[C:]]]
    )[None, :]  # [1, 258]
    wo_ext = np.concatenate([wo_top, wo_bot], axis=0).astype(BF16)  # [513, 258]

    ones_row = np.ones((1, N), BF16)
    in_maps = []
    for b in range(B):
        psicol = np.empty((N, RK * H), np.float32)
        rhof = np.empty((N, H), np.float32)
        for hh in range(H):
            rho, psi = _fit_rank2(sl_all[b, hh], sr_all[b, hh])
            rhof[:, hh] = rho
            for k in range(RK):
                psicol[:, k * H + hh] = psi[:, k]
        xtb = np.concatenate([x[b].T.astype(BF16), ones_row], axis=0)  # [257, 1024]
        mb = np.where(adj[b].T > 0, np.float32(1.0), np.float32(0.0)).astype(BF16)
        in_maps.append(
            {
                "xt": np.ascontiguousarray(xtb),
                "xs": np.ascontiguousarray(x[b]),
                "msk": np.ascontiguousarray(mb),
                "wp": wp,
                "psicol": psicol.astype(BF16),
                "rhof": rhof,
                "wo": wo_ext,
            }
        )
    return in_maps


def kernel(**inputs) -> np.ndarray:
    from concourse.bass_utils import run_bass_kernel_spmd

    nc = get_program()
    in_maps = make_in_maps(**inputs)
    res = run_bass_kernel_spmd(nc, in_maps, core_ids=list(range(B)))
    return np.stack([res.results[b]["out"] for b in range(B)], axis=0)


# revision 18
# speedup vs baseline: 1.0855x; 1.0855x over previous
"""Trainium2 Bass kernel for a 2-layer GAT (B=8, N=1024, F=256, D=64, H=8, C=256).

Sharding: data-parallel over batch — one batch element per NeuronCore (8 cores).

Layer-1 attention uses a host-fitted rank-2 separable factorization of the
scalar kernel g(s) = exp(LeakyReLU(s)) evaluated at s = sl_i + sr_j:

    g(sl_i + sr_j) ~= phi0(sl_i) psi0(sr_j) + phi1(sl_i) psi1(sr_j)

(per batch, per head, SVD of g on the realized [sl]x[sr] box). The masked
softmax aggregation then needs NO N^2 elementwise work:

    num_i = phi0_i (M @ (psi0 . h))_i + phi1_i (M @ (psi1 . h))_i
    Z_i   = phi0_i (M @ psi0)_i      + phi1_i (M @ psi1)_i
    attn-out_i = num_i / Z_i                     (phi0 cancels; rho=phi1/phi0)

so layer-1 is mask matmuls (lhsT = adjT chunk, shared across heads/ranks)
over value blocks psi_k.h. psi/rho ship as tiny per-node columns and are
broadcast-expanded on device (keeps startup DMA small). Layer 2 (single
head, C=256) keeps the exact masked-exp sweep; tl/tr are produced early via
2-col matmuls so the g-projection can overlap the serial sweep.
"""

import numpy as np
import ml_dtypes
from contextlib import ExitStack

BF16 = ml_dtypes.bfloat16
B, N, F, D, H, C = 8, 1024, 256, 64, 8, 256
HD = H * D  # 512
RK = 2  # separable rank for layer-1 attention
ALPHA = 0.2

_CACHE = {}


def _build_program():
    import concourse.bacc as bacc
    import concourse.bass as bass
    import concourse.mybir as mybir
    from concourse.tile import TileContext
    from concourse.masks import make_identity

    dt = mybir.dt
    Alu = mybir.AluOpType
    Act = mybir.ActivationFunctionType

    nc = bacc.Bacc()

    xt = nc.declare_dram_parameter("xt", [F + 1, N], dt.bfloat16, isOutput=False)
    xs = nc.declare_dram_parameter("xs", [N, F], dt.float32, isOutput=False)
    msk = nc.declare_dram_parameter("msk", [N, N], dt.bfloat16, isOutput=False)
    wp = nc.declare_dram_parameter("wp", [F + 1, HD], dt.bfloat16, isOutput=False)
    psicol = nc.declare_dram_parameter(
        "psicol", [N, RK * H], dt.bfloat16, isOutput=False
    )
    psirep = nc.declare_dram_parameter(
        "psirep", [N, RK * HD], dt.bfloat16, isOutput=False
    )
    rhorep = nc.declare_dram_parameter(
        "rhorep", [N, HD], dt.bfloat16, isOutput=False
    )
    rhof = nc.declare_dram_parameter("rhof", [N, H], dt.float32, isOutput=False)
    wo = nc.declare_dram_parameter("wo", [HD + 1, C + 2], dt.bfloat16, isOutput=False)
    out_d = nc.declare_dram_parameter("out", [N, C], dt.float32, isOutput=True)

    rows_d = nc.dram_tensor("rows_bounce", [2, N], dt.bfloat16)

    NCH = N // 128  # 8 chunks of 128 nodes

    def bcast128(row_ap):
        # [1, N] DRAM row -> [128, N] partition-broadcast read for DMA
        return bass.AP(
            tensor=row_ap.tensor,
            offset=row_ap.offset,
            ap=[[0, 128]] + list(row_ap.ap),
        )

    with TileContext(nc) as tc:
        with ExitStack() as ctx:
            cons = ctx.enter_context(tc.tile_pool(name="cons", bufs=1))
            bc = ctx.enter_context(tc.tile_pool(name="bc", bufs=2))
            eb = ctx.enter_context(tc.tile_pool(name="eb", bufs=1))
            tb = ctx.enter_context(tc.tile_pool(name="tb", bufs=1))
            wk = ctx.enter_context(tc.tile_pool(name="wk", bufs=3))
            sm = ctx.enter_context(tc.tile_pool(name="sm", bufs=3))
            pa0p = ctx.enter_context(tc.tile_pool(name="pa0", bufs=2, space="PSUM"))
            pa1p = ctx.enter_context(tc.tile_pool(name="pa1", bufs=2, space="PSUM"))
            pzp = ctx.enter_context(tc.tile_pool(name="pzp", bufs=1, space="PSUM"))
            pm2 = ctx.enter_context(tc.tile_pool(name="pm2", bufs=2, space="PSUM"))
            ptp = ctx.enter_context(tc.tile_pool(name="ptp", bufs=1, space="PSUM"))

            # ---------- constants ----------
            ident_f = cons.tile([128, 128], dt.float32)
            make_identity(nc, ident_f[:, :])
            ident_b = cons.tile([128, 128], dt.bfloat16)
            make_identity(nc, ident_b[:, :])

            # ---------- DMAs: critical path first (sync q), bulk on gpsimd q
            xt_sb = cons.tile([128, 2 * N], dt.bfloat16)
            nc.sync.dma_start(out=xt_sb[:, 0:N], in_=xt[0:128, :])
            nc.sync.dma_start(out=xt_sb[:, N : 2 * N], in_=xt[128:256, :])
            xt_one = cons.tile([1, N], dt.bfloat16)
            nc.sync.dma_start(out=xt_one[:, :], in_=xt[256:257, :])

            wp_sb = cons.tile([128, 2 * HD], dt.bfloat16)
            nc.sync.dma_start(out=wp_sb[:, 0:HD], in_=wp[0:128, :])
            nc.sync.dma_start(out=wp_sb[:, HD : 2 * HD], in_=wp[128:256, :])
            wp_one = cons.tile([1, HD], dt.bfloat16)
            nc.sync.dma_start(out=wp_one[:, :], in_=wp[256:257, :])

            psicol_sb = cons.tile([128, NCH * RK * H], dt.bfloat16)
            nc.sync.dma_start(
                out=psicol_sb[:, :].rearrange("p (n q) -> p n q", q=RK * H),
                in_=psicol[:, :].rearrange("(n p) q -> p n q", p=128),
            )
            rhof_sb = cons.tile([128, NCH * H], dt.float32)
            nc.sync.dma_start(
                out=rhof_sb[:, :].rearrange("p (n q) -> p n q", q=H),
                in_=rhof[:, :].rearrange("(n p) q -> p n q", p=128),
            )
            # psirep (2MB) on the sync queue after the small criticals —
            # V-prep consumes it from ~12us in
            psirep_sb = cons.tile([128, NCH * RK * HD], dt.bfloat16)
            nc.sync.dma_start(
                out=psirep_sb[:, :].rearrange("p (n q) -> p n q", q=RK * HD),
                in_=psirep[:, :].rearrange("(n p) q -> p n q", p=128),
            )

            # bulk tensors on the gpsimd queue in need-time order
            msk_sb = cons.tile([128, NCH * N], dt.bfloat16)
            for c in range(NCH):
                nc.gpsimd.dma_start(
                    out=msk_sb[:, c * N : (c + 1) * N],
                    in_=msk[c * 128 : (c + 1) * 128, :],
                )

            wo_sb = cons.tile([128, 4 * (C + 2)], dt.bfloat16)
            for k in range(4):
                nc.gpsimd.dma_start(
                    out=wo_sb[:, k * (C + 2) : (k + 1) * (C + 2)],
                    in_=wo[k * 128 : (k + 1) * 128, :],
                )
            wo_one = cons.tile([1, C + 2], dt.bfloat16)
            nc.gpsimd.dma_start(out=wo_one[:, :], in_=wo[HD : HD + 1, :])

            rhorep_sb = cons.tile([128, NCH * HD], dt.bfloat16)
            nc.gpsimd.dma_start(
                out=rhorep_sb[:, :].rearrange("p (n q) -> p n q", q=HD),
                in_=rhorep[:, :].rearrange("(n p) q -> p n q", p=128),
            )

            # ---------- phase 1: h = x@W_all ; V = psi_k . h ----------
            hx = cons.tile([128, NCH * HD], dt.bfloat16)
            v_sb = cons.tile([128, NCH * RK * HD], dt.bfloat16)
            for n in range(NCH):
                hp_pool = pa0p if n % 2 == 0 else pa1p
                ph = hp_pool.tile(
                    [128, HD], dt.float32, tag="a0" if n % 2 == 0 else "a1"
                )
                for k in range(2):
                    lt = xt_sb[:, k * N + n * 128 : k * N + n * 128 + 128]
                    nc.tensor.matmul(
                        ph[:, :], lt, wp_sb[:, k * HD : (k + 1) * HD],
                        start=(k == 0), stop=False,
                    )
                lt1 = xt_one[:, n * 128 : n * 128 + 128]
                nc.tensor.matmul(ph[:, :], lt1, wp_one[:, :], start=False, stop=True)
                nc.scalar.activation(
                    hx[:, n * HD : (n + 1) * HD], ph[:, :], Act.Copy
                )
                for k in range(RK):
                    base = n * RK * HD + k * HD
                    nc.vector.tensor_tensor(
                        out=v_sb[:, base : base + HD],
                        in0=hx[:, n * HD : (n + 1) * HD],
                        in1=psirep_sb[:, base : base + HD],
                        op=Alu.mult,
                    )

            # ---------- phase 2: L1 agg -> z -> zT -> tl/tr ----------
            z_sb = cons.tile([128, NCH * HD], dt.bfloat16)
            zt_sb = cons.tile([128, 4 * N], dt.bfloat16)
            zt_one = cons.tile([1, N], dt.bfloat16)
            nc.vector.memset(zt_one[:, :], 1.0)
            gx = cons.tile([128, NCH * 260], dt.bfloat16)
            nc.vector.memset(
                gx[:, :].rearrange("p (n s) -> p n s", s=260)[:, :, 256:257], 1.0
            )
            glgr = cons.tile([128, NCH * 2], dt.float32)

            # pz_all: [Z(even) 16 | Z(odd) 16 | tlr(even) 2 | tlr(odd) 2] x2 halves
            PZW = 2 * RK * H + 4  # 36
            pz_all = pzp.tile([128, 2 * PZW], dt.float32, tag="az")
            for ip in range(NCH // 2):
                po_ = (ip % 2) * PZW
                pa = []
                for par in range(2):
                    ic = 2 * ip + par
                    pa0 = pa0p.tile([128, HD], dt.float32, tag="a0")
                    pa1 = pa1p.tile([128, HD], dt.float32, tag="a1")
                    pa.append((pa0, pa1))
                    pzc = pz_all[:, po_ + par * RK * H : po_ + (par + 1) * RK * H]
                    for jc in range(NCH):
                        w = msk_sb[:, jc * N + ic * 128 : jc * N + ic * 128 + 128]
                        st = jc == 0
                        sp = jc == NCH - 1
                        nc.tensor.matmul(
                            pa0[:, :], w,
                            v_sb[:, jc * RK * HD : jc * RK * HD + HD],
                            start=st, stop=sp,
                        )
                        nc.tensor.matmul(
                            pa1[:, :], w,
                            v_sb[:, jc * RK * HD + HD : jc * RK * HD + 2 * HD],
                            start=st, stop=sp,
                        )
                        nc.tensor.matmul(
                            pzc, w,
                            psicol_sb[:, jc * RK * H : (jc + 1) * RK * H],
                            start=st, stop=sp,
                        )
                # exits to bf16 (pairwise [128, 1024] working set)
                n0 = wk.tile([128, 2 * HD], dt.bfloat16, tag="n0")
                n1 = wk.tile([128, 2 * HD], dt.bfloat16, tag="n1")
                for par in range(2):
                    nc.scalar.activation(
                        n0[:, par * HD : (par + 1) * HD], pa[par][0][:, :], Act.Copy
                    )
                    nc.scalar.activation(
                        n1[:, par * HD : (par + 1) * HD], pa[par][1][:, :], Act.Copy
                    )
                # Z = pz[k0] + rho . pz[k1]  (fp32), both ics at once
                pzv = pz_all[:, po_ : po_ + 2 * RK * H].rearrange(
                    "p (i k h) -> p i k h", i=2, k=RK
                )
                rhob = rhof_sb[:, 2 * ip * H : (2 * ip + 2) * H]
                zt1 = wk.tile([128, 4 * H], dt.float32, tag="zt1")
                nc.vector.tensor_tensor(
                    out=zt1[:, 0 : 2 * H].rearrange("p (i h) -> p i h", i=2),
                    in0=pzv[:, :, 1, :],
                    in1=rhob.rearrange("p (i h) -> p i h", i=2),
                    op=Alu.mult,
                )
                nc.vector.tensor_tensor(
                    out=zt1[:, 2 * H : 4 * H].rearrange("p (i h) -> p i h", i=2),
                    in0=zt1[:, 0 : 2 * H].rearrange("p (i h) -> p i h", i=2),
                    in1=pzv[:, :, 0, :], op=Alu.add,
                )
                rz = wk.tile([128, 2 * H], dt.float32, tag="rz")
                nc.vector.reciprocal(
                    rz[:, :].rearrange("p (h s) -> p h s", s=1),
                    zt1[:, 2 * H : 4 * H].rearrange("p (h s) -> p h s", s=1),
                )
                # num = n0 + rhorep . n1 ; hh = num . rzrep
                num = wk.tile([128, 2 * HD], dt.bfloat16, tag="num")
                nc.vector.tensor_tensor(
                    out=num[:, :], in0=n1[:, :],
                    in1=rhorep_sb[:, 2 * ip * HD : (2 * ip + 2) * HD],
                    op=Alu.mult,
                )
                nc.vector.tensor_tensor(
                    out=num[:, :], in0=num[:, :], in1=n0[:, :], op=Alu.add
                )
                rzrep = wk.tile([128, 2 * HD], dt.bfloat16, tag="rzrep")
                nc.vector.tensor_copy(
                    out=rzrep[:, :].rearrange("p (h s) -> p h s", s=D),
                    in_=rz[:, :]
                    .rearrange("p (h s) -> p h s", s=1)
                    .to_broadcast([128, 2 * H, D]),
                )
                hh = wk.tile([128, 2 * HD], dt.bfloat16, tag="hh")
                nc.vector.tensor_tensor(
                    out=hh[:, :], in0=num[:, :], in1=rzrep[:, :], op=Alu.mult
                )
                # ELU(x) = max(x, min(exp(x)-1, 0))
                ee = wk.tile([128, 2 * HD], dt.bfloat16, tag="ee")
                nc.scalar.activation(ee[:, :], hh[:, :], Act.Exp)
                r1 = wk.tile([128, 2 * HD], dt.bfloat16, tag="r1")
                nc.vector.tensor_scalar(
                    out=r1[:, :], in0=ee[:, :], scalar1=-1.0, scalar2=0.0,
                    op0=Alu.add, op1=Alu.min,
                )
                nc.vector.tensor_tensor(
                    out=z_sb[:, 2 * ip * HD : (2 * ip + 2) * HD],
                    in0=hh[:, :], in1=r1[:, :], op=Alu.max,
                )

                # zT (8 transposed 128x128 blocks, kc-major then parity)
                pzi = pm2.tile([128, 8 * 128], dt.bfloat16, tag="mm2")
                for kc in range(4):
                    for par in range(2):
                        ic = 2 * ip + par
                        nc.tensor.transpose(
                            pzi[:, (kc * 2 + par) * 128 : (kc * 2 + par + 1) * 128],
                            z_sb[:, ic * HD + kc * 128 : ic * HD + kc * 128 + 128],
                            ident_b[:, :],
                        )
                nc.vector.tensor_copy(
                    out=zt_sb[:, :]
                    .rearrange("p (kc n) -> p kc n", n=N)[
                        :, :, 2 * ip * 128 : 2 * ip * 128 + 256
                    ],
                    in_=pzi[:, :].rearrange("p (kc s) -> p kc s", s=256),
                )

                # tl/tr via 2-col matmuls (u_l/u_r cols of wo), per ic
                for par in range(2):
                    ic = 2 * ip + par
                    ptl = pz_all[:, po_ + 32 + 2 * par : po_ + 34 + 2 * par]
                    for kc in range(4):
                        nc.tensor.matmul(
                            ptl,
                            zt_sb[:, kc * N + ic * 128 : kc * N + ic * 128 + 128],
                            wo_sb[:, kc * (C + 2) + C : kc * (C + 2) + C + 2],
                            start=(kc == 0), stop=False,
                        )
                    nc.tensor.matmul(
                        ptl, zt_one[:, ic * 128 : ic * 128 + 128],
                        wo_one[:, C : C + 2], start=False, stop=True,
                    )
                nc.vector.tensor_copy(
                    out=glgr[:, ip * 4 : ip * 4 + 4],
                    in_=pz_all[:, po_ + 32 : po_ + 36],
                )
                gb16 = wk.tile([128, 4], dt.bfloat16, tag="gb16")
                nc.vector.tensor_copy(
                    out=gb16[:, :], in_=glgr[:, ip * 4 : ip * 4 + 4]
                )
                pt2 = ptp.tile([4, 128], dt.bfloat16, tag="tp")
                nc.tensor.transpose(pt2[:, :], gb16[:, :], ident_b[:, :])
                gr2 = wk.tile([4, 128], dt.bfloat16, tag="gr2")
                nc.vector.tensor_copy(out=gr2[:, :], in_=pt2[:, :])
                for par in range(2):
                    ic = 2 * ip + par
                    nc.sync.dma_start(
                        out=rows_d[0:2, ic * 128 : (ic + 1) * 128],
                        in_=gr2[2 * par : 2 * par + 2, :],
                    )

            # ---------- phase 3a: L2 sweep prep (tl row broadcast) ----------
            CA = 3  # chunks LeakyReLU'd by ACT (Prelu bias trick); rest DVE
            glb = bc.tile([128, N], dt.bfloat16, tag="slb")
            nc.sync.dma_start(out=glb[:, :], in_=bcast128(rows_d[0:1, :]))
            e2 = eb.tile([128, NCH * N], dt.bfloat16, tag="e")
            for c in range(CA):
                nc.scalar.activation(
                    e2[:, c * N : (c + 1) * N], glb[:, :], Act.Prelu,
                    bias=glgr[:, c * 2 + 1 : c * 2 + 2], scale=1.0, alpha=ALPHA,
                )
            s = CA * N
            w_ = (NCH - CA) * N
            for c in range(CA, NCH):
                nc.vector.tensor_scalar(
                    out=e2[:, c * N : (c + 1) * N], in0=glb[:, :],
                    scalar1=glgr[:, c * 2 + 1 : c * 2 + 2], scalar2=None,
                    op0=Alu.add,
                )
            t = tb.tile([128, (NCH - CA) * N], dt.bfloat16, tag="t")
            nc.vector.tensor_scalar(
                out=t[:, 0:w_], in0=e2[:, s : s + w_], scalar1=ALPHA,
                scalar2=None, op0=Alu.mult,
            )
            nc.vector.tensor_tensor(
                out=e2[:, s : s + w_], in0=t[:, 0:w_], in1=e2[:, s : s + w_],
                op=Alu.max,
            )

            # ---------- phase 3b: g-projection (overlaps the sweep) ----------
            for ic in range(NCH):
                pg = pm2.tile([128, C + 2], dt.float32, tag="mm2")
                for kc in range(4):
                    nc.tensor.matmul(
                        pg[:, :],
                        zt_sb[:, kc * N + ic * 128 : kc * N + ic * 128 + 128],
                        wo_sb[:, kc * (C + 2) : (kc + 1) * (C + 2)],
                        start=(kc == 0), stop=False,
                    )
                nc.tensor.matmul(
                    pg[:, :], zt_one[:, ic * 128 : ic * 128 + 128], wo_one[:, :],
                    start=False, stop=True,
                )
                nc.vector.tensor_copy(
                    out=gx[:, ic * 260 : ic * 260 + C], in_=pg[:, 0:C]
                )

            # exp + mask in quarters for ACT/DVE pipelining
            QN = NCH * N // 4
            for q in range(4):
                nc.scalar.activation(
                    e2[:, q * QN : (q + 1) * QN], e2[:, q * QN : (q + 1) * QN],
                    Act.Exp,
                )
                nc.vector.tensor_tensor(
                    out=e2[:, q * QN : (q + 1) * QN],
                    in0=e2[:, q * QN : (q + 1) * QN],
                    in1=msk_sb[:, q * QN : (q + 1) * QN], op=Alu.mult,
                )

            # ---------- phase 4: L2 aggregation + ELU + residual ----------
            for ic in range(NCH):
                po = pa0p.tile([128, HD], dt.float32, tag="a0")
                for jc in range(NCH):
                    nc.tensor.matmul(
                        po[:, 0 : C + 1],
                        e2[:, jc * N + ic * 128 : jc * N + ic * 128 + 128],
                        gx[:, jc * 260 : jc * 260 + C + 1],
                        start=(jc == 0), stop=(jc == NCH - 1),
                    )
                rz2 = sm.tile([128, 1], dt.float32, tag="rz2")
                nc.vector.reciprocal(rz2[:, :], po[:, C : C + 1])
                y = sm.tile([128, C], dt.bfloat16, tag="y")
                nc.vector.tensor_scalar(
                    out=y[:, :], in0=po[:, 0:C], scalar1=rz2[:, :], scalar2=None,
                    op0=Alu.mult,
                )
                e3 = sm.tile([128, C], dt.bfloat16, tag="e3")
                nc.scalar.activation(e3[:, :], y[:, :], Act.Exp)
                r2 = sm.tile([128, C], dt.bfloat16, tag="r2")
                nc.vector.tensor_scalar(
                    out=r2[:, :], in0=e3[:, :], scalar1=-1.0, scalar2=0.0,
                    op0=Alu.add, op1=Alu.min,
                )
                el = sm.tile([128, C], dt.bfloat16, tag="el")
                nc.vector.tensor_tensor(
                    out=el[:, :], in0=y[:, :], in1=r2[:, :], op=Alu.max
                )
                xs5 = sm.tile([128, F], dt.float32, tag="xs5")
                nc.sync.dma_start(
                    out=xs5[:, :], in_=xs[ic * 128 : (ic + 1) * 128, :]
                )
                ofin = sm.tile([128, C], dt.float32, tag="ofin")
                nc.vector.tensor_tensor(
                    out=ofin[:, :], in0=el[:, :], in1=xs5[:, :], op=Alu.add,
                )
                nc.sync.dma_start(
                    out=out_d[ic * 128 : (ic + 1) * 128, :], in_=ofin[:, :]
                )

    nc.compile()
    return nc


def get_program():
    if "nc" not in _CACHE:
        _CACHE["nc"] = _build_program()
    return _CACHE["nc"]


def _fit_rank2(sl, sr, ngrid=257):
    """Fit g(x+y)=exp(LeakyReLU(x+y)) ~= sum_k phi_k(x) psi_k(y), rank RK,
    on the realized box. Returns (rho[N] fp32, psi[N, RK] fp32)."""
    pad_x = 1e-3 * (sl.max() - sl.min()) + 1e-6
    pad_y = 1e-3 * (sr.max() - sr.min()) + 1e-6
    xs = np.linspace(sl.min() - pad_x, sl.max() + pad_x, ngrid)
    ys = np.linspace(sr.min() - pad_y, sr.max() + pad_y, ngrid)
    ss = xs[:, None] + ys[None, :]
    G = np.exp(np.where(ss >= 0, ss, ALPHA * ss))
    U, S, Vt = np.linalg.svd(G, full_matrices=False)
    phi_g = U[:, :RK] * S[:RK]
    psi_g = Vt[:RK].T
    if phi_g[:, 0].mean() < 0:
        phi_g[:, 0] *= -1.0
        psi_g[:, 0] *= -1.0
    phi = np.stack([np.interp(sl, xs, phi_g[:, k]) for k in range(RK)], axis=1)
    psi = np.stack([np.interp(sr, ys, psi_g[:, k]) for k in range(RK)], axis=1)
    assert np.all(phi[:, 0] > 0), "phi0 must be positive"
    rho = phi[:, 1] / phi[:, 0]
    return rho.astype(np.float32), psi.astype(np.float32)


def make_in_maps(x, adj, W, Wb, a, ab, Wo, Wob, ao, aob):
    x = np.asarray(x, np.float32)
    adj = np.asarray(adj)
    W = np.asarray(W, np.float32)
    Wb = np.asarray(Wb, np.float32)
    a = np.asarray(a, np.float32)
    ab = np.asarray(ab, np.float32)
    Wo = np.asarray(Wo, np.float32)
    Wob = np.asarray(Wob, np.float32)
    ao = np.asarray(ao, np.float32)
    aob = np.asarray(aob, np.float32)

    # W_all[f, h*D+d] = W[h, f, d];  Wb row flattened the same way
    W_all = W.transpose(1, 0, 2).reshape(F, HD)
    wb_row = Wb.reshape(1, HD)
    wp = np.concatenate([W_all, wb_row], axis=0).astype(BF16)  # [257, 512]

    # sl/sr per-node linear maps of x, folded on the host (fp32)
    V_l = np.einsum("hfd,hd->fh", W, a[:, :D]).astype(np.float32)
    V_r = np.einsum("hfd,hd->fh", W, a[:, D:]).astype(np.float32)
    const_l = (Wb * a[:, :D]).sum(1) + ab  # [H]
    const_r = (Wb * a[:, D:]).sum(1)
    sl_all = np.einsum("bnf,fh->bhn", x, V_l) + const_l[None, :, None]  # [B,H,N]
    sr_all = np.einsum("bnf,fh->bhn", x, V_r) + const_r[None, :, None]  # [B,H,N]

    u_l = Wo @ ao[:C]  # [512]
    u_r = Wo @ ao[C:]
    wo_top = np.concatenate([Wo, u_l[:, None], u_r[:, None]], axis=1)  # [512, 258]
    wo_bot = np.concatenate(
        [Wob, [Wob @ ao[:C] + aob], [Wob @ ao[C:]]]
    )[None, :]  # [1, 258]
    wo_ext = np.concatenate([wo_top, wo_bot], axis=0).astype(BF16)  # [513, 258]

    ones_row = np.ones((1, N), BF16)
    in_maps = []
    for b in range(B):
        psicol = np.empty((N, RK * H), np.float32)
        rhof = np.empty((N, H), np.float32)
        for hh in range(H):
            rho, psi = _fit_rank2(sl_all[b, hh], sr_all[b, hh])
            rhof[:, hh] = rho
            for k in range(RK):
                psicol[:, k * H + hh] = psi[:, k]
        psicol_b = psicol.astype(BF16)
        psirep_b = np.repeat(
            psicol_b.reshape(N, RK * H), D, axis=1
        )  # [N, RK*H*D], bf16 values replicated per head block
        rhorep_b = np.repeat(rhof.astype(BF16), D, axis=1)  # [N, HD]
        xtb = np.concatenate([x[b].T.astype(BF16), ones_row], axis=0)  # [257, 1024]
        mb = np.where(adj[b].T > 0, np.float32(1.0), np.float32(0.0)).astype(BF16)
        in_maps.append(
            {
                "xt": np.ascontiguousarray(xtb),
                "xs": np.ascontiguousarray(x[b]),
                "msk": np.ascontiguousarray(mb),
                "wp": wp,
                "psicol": psicol_b,
                "psirep": np.ascontiguousarray(psirep_b),
                "rhorep": np.ascontiguousarray(rhorep_b),
                "rhof": rhof,
                "wo": wo_ext,
            }
        )
    return in_maps


def kernel(**inputs) -> np.ndarray:
    from concourse.bass_utils import run_bass_kernel_spmd

    nc = get_program()
    in_maps = make_in_maps(**inputs)
    res = run_bass_kernel_spmd(nc, in_maps, core_ids=list(range(B)))
    return np.stack([res.results[b]["out"] for b in range(B)], axis=0)


# revision 21
# speedup vs baseline: 1.1578x; 1.0666x over previous
"""Trainium2 Bass kernel for a 2-layer GAT (B=8, N=1024, F=256, D=64, H=8, C=256).

Sharding: data-parallel over batch — one batch element per NeuronCore (8 cores).

Layer-1 attention uses a host-fitted rank-2 separable factorization of the
scalar kernel g(s) = exp(LeakyReLU(s)) evaluated at s = sl_i + sr_j:

    g(sl_i + sr_j) ~= phi0(sl_i) psi0(sr_j) + phi1(sl_i) psi1(sr_j)

(per batch, per head, SVD of g on the realized [sl]x[sr] box). The masked
softmax aggregation then needs NO N^2 elementwise work:

    num_i = phi0_i (M @ (psi0 . h))_i + phi1_i (M @ (psi1 . h))_i
    Z_i   = phi0_i (M @ psi0)_i      + phi1_i (M @ psi1)_i
    attn-out_i = num_i / Z_i                     (phi0 cancels; rho=phi1/phi0)

so layer-1 is mask matmuls (lhsT = adjT chunk, shared across heads/ranks)
over value blocks psi_k.h. psi/rho ship as tiny per-node columns and are
broadcast-expanded on device (keeps startup DMA small). Layer 2 (single
head, C=256) keeps the exact masked-exp sweep; tl/tr are produced early via
2-col matmuls so the g-projection can overlap the serial sweep.
"""

import numpy as np
import ml_dtypes
from contextlib import ExitStack

BF16 = ml_dtypes.bfloat16
B, N, F, D, H, C = 8, 1024, 256, 64, 8, 256
HD = H * D  # 512
RK = 2  # separable rank for layer-1 attention
ALPHA = 0.2

_CACHE = {}


def _build_program():
    import concourse.bacc as bacc
    import concourse.bass as bass
    import concourse.mybir as mybir
    from concourse.tile import TileContext
    from concourse.masks import make_identity

    dt = mybir.dt
    Alu = mybir.AluOpType
    Act = mybir.ActivationFunctionType

    nc = bacc.Bacc()

    xt = nc.declare_dram_parameter("xt", [F + 1, N], dt.bfloat16, isOutput=False)
    xs = nc.declare_dram_parameter("xs", [N, F], dt.float32, isOutput=False)
    msk = nc.declare_dram_parameter("msk", [N, N], dt.bfloat16, isOutput=False)
    wp = nc.declare_dram_parameter("wp", [F + 1, HD], dt.bfloat16, isOutput=False)
    psicol = nc.declare_dram_parameter(
        "psicol", [N, RK * H], dt.bfloat16, isOutput=False
    )
    psirep = nc.declare_dram_parameter(
        "psirep", [N, RK * HD], dt.bfloat16, isOutput=False
    )
    rhorep = nc.declare_dram_parameter(
        "rhorep", [N, HD], dt.bfloat16, isOutput=False
    )
    rhof = nc.declare_dram_parameter("rhof", [N, H], dt.float32, isOutput=False)
    wo = nc.declare_dram_parameter("wo", [HD + 1, C + 2], dt.bfloat16, isOutput=False)
    out_d = nc.declare_dram_parameter("out", [N, C], dt.float32, isOutput=True)

    rows_d = nc.dram_tensor("rows_bounce", [2, N], dt.bfloat16)

    NCH = N // 128  # 8 chunks of 128 nodes

    def bcast128(row_ap):
        # [1, N] DRAM row -> [128, N] partition-broadcast read for DMA
        return bass.AP(
            tensor=row_ap.tensor,
            offset=row_ap.offset,
            ap=[[0, 128]] + list(row_ap.ap),
        )

    with TileContext(nc) as tc:
        with ExitStack() as ctx:
            cons = ctx.enter_context(tc.tile_pool(name="cons", bufs=1))
            bc = ctx.enter_context(tc.tile_pool(name="bc", bufs=2))
            eb = ctx.enter_context(tc.tile_pool(name="eb", bufs=1))
            tb = ctx.enter_context(tc.tile_pool(name="tb", bufs=1))
            wk = ctx.enter_context(tc.tile_pool(name="wk", bufs=3))
            sm = ctx.enter_context(tc.tile_pool(name="sm", bufs=3))
            pa0p = ctx.enter_context(tc.tile_pool(name="pa0", bufs=2, space="PSUM"))
            pa1p = ctx.enter_context(tc.tile_pool(name="pa1", bufs=2, space="PSUM"))
            pzp = ctx.enter_context(tc.tile_pool(name="pzp", bufs=1, space="PSUM"))
            pm2 = ctx.enter_context(tc.tile_pool(name="pm2", bufs=2, space="PSUM"))
            ptp = ctx.enter_context(tc.tile_pool(name="ptp", bufs=1, space="PSUM"))

            # ---------- constants ----------
            ident_f = cons.tile([128, 128], dt.float32)
            make_identity(nc, ident_f[:, :])
            ident_b = cons.tile([128, 128], dt.bfloat16)
            make_identity(nc, ident_b[:, :])

            # ---------- DMAs: critical path first (sync q), bulk on gpsimd q
            xt_sb = cons.tile([128, 2 * N], dt.bfloat16)
            nc.sync.dma_start(out=xt_sb[:, 0:N], in_=xt[0:128, :])
            nc.sync.dma_start(out=xt_sb[:, N : 2 * N], in_=xt[128:256, :])
            xt_one = cons.tile([1, N], dt.bfloat16)
            nc.sync.dma_start(out=xt_one[:, :], in_=xt[256:257, :])

            wp_sb = cons.tile([128, 2 * HD], dt.bfloat16)
            nc.sync.dma_start(out=wp_sb[:, 0:HD], in_=wp[0:128, :])
            nc.sync.dma_start(out=wp_sb[:, HD : 2 * HD], in_=wp[128:256, :])
            wp_one = cons.tile([1, HD], dt.bfloat16)
            nc.sync.dma_start(out=wp_one[:, :], in_=wp[256:257, :])

            psicol_sb = cons.tile([128, NCH * RK * H], dt.bfloat16)
            nc.sync.dma_start(
                out=psicol_sb[:, :].rearrange("p (n q) -> p n q", q=RK * H),
                in_=psicol[:, :].rearrange("(n p) q -> p n q", p=128),
            )
            rhof_sb = cons.tile([128, NCH * H], dt.float32)
            nc.sync.dma_start(
                out=rhof_sb[:, :].rearrange("p (n q) -> p n q", q=H),
                in_=rhof[:, :].rearrange("(n p) q -> p n q", p=128),
            )
            # psirep (2MB) on the sync queue after the small criticals —
            # V-prep consumes it from ~12us in
            psirep_sb = cons.tile([128, NCH * RK * HD], dt.bfloat16)
            nc.sync.dma_start(
                out=psirep_sb[:, :].rearrange("p (n q) -> p n q", q=RK * HD),
                in_=psirep[:, :].rearrange("(n p) q -> p n q", p=128),
            )

            # bulk tensors on the gpsimd queue in need-time order
            msk_sb = cons.tile([128, NCH * N], dt.bfloat16)
            for c in range(NCH):
                nc.gpsimd.dma_start(
                    out=msk_sb[:, c * N : (c + 1) * N],
                    in_=msk[c * 128 : (c + 1) * 128, :],
                )

            wo_sb = cons.tile([128, 4 * (C + 2)], dt.bfloat16)
            for k in range(4):
                nc.gpsimd.dma_start(
                    out=wo_sb[:, k * (C + 2) : (k + 1) * (C + 2)],
                    in_=wo[k * 128 : (k + 1) * 128, :],
                )
            wo_one = cons.tile([1, C + 2], dt.bfloat16)
            nc.gpsimd.dma_start(out=wo_one[:, :], in_=wo[HD : HD + 1, :])

            rhorep_sb = cons.tile([128, NCH * HD], dt.bfloat16)
            nc.gpsimd.dma_start(
                out=rhorep_sb[:, :].rearrange("p (n q) -> p n q", q=HD),
                in_=rhorep[:, :].rearrange("(n p) q -> p n q", p=128),
            )

            # ---------- phase 1: h = x@W_all ; V = psi_k . h ----------
            hx = cons.tile([128, NCH * HD], dt.bfloat16)
            v_sb = cons.tile([128, NCH * RK * HD], dt.bfloat16)
            for n in range(NCH):
                hp_pool = pa0p if n % 2 == 0 else pa1p
                ph = hp_pool.tile(
                    [128, HD], dt.float32, tag="a0" if n % 2 == 0 else "a1"
                )
                for k in range(2):
                    lt = xt_sb[:, k * N + n * 128 : k * N + n * 128 + 128]
                    nc.tensor.matmul(
                        ph[:, :], lt, wp_sb[:, k * HD : (k + 1) * HD],
                        start=(k == 0), stop=False,
                    )
                lt1 = xt_one[:, n * 128 : n * 128 + 128]
                nc.tensor.matmul(ph[:, :], lt1, wp_one[:, :], start=False, stop=True)
                nc.scalar.activation(
                    hx[:, n * HD : (n + 1) * HD], ph[:, :], Act.Copy
                )
                for k in range(RK):
                    base = n * RK * HD + k * HD
                    nc.vector.tensor_tensor(
                        out=v_sb[:, base : base + HD],
                        in0=hx[:, n * HD : (n + 1) * HD],
                        in1=psirep_sb[:, base : base + HD],
                        op=Alu.mult,
                    )

            # ---------- phase 2: L1 agg -> z -> zT -> tl/tr ----------
            z_sb = cons.tile([128, NCH * HD], dt.bfloat16)
            zt_sb = cons.tile([128, 4 * N], dt.bfloat16)
            zt_one = cons.tile([1, N], dt.bfloat16)
            nc.vector.memset(zt_one[:, :], 1.0)
            gx = cons.tile([128, NCH * 260], dt.bfloat16)
            nc.vector.memset(
                gx[:, :].rearrange("p (n s) -> p n s", s=260)[:, :, 256:257], 1.0
            )
            glgr = cons.tile([128, NCH * 2], dt.float32)
            glb = cons.tile([128, N], dt.bfloat16)

            # pz_all: [Z(even) 16 | Z(odd) 16 | tlr(even) 2 | tlr(odd) 2] x2 halves
            PZW = 2 * RK * H + 4  # 36
            pz_all = pzp.tile([128, 2 * PZW], dt.float32, tag="az")
            for ip in range(NCH // 2):
                po_ = (ip % 2) * PZW
                pa = []
                for par in range(2):
                    ic = 2 * ip + par
                    pa0 = pa0p.tile([128, HD], dt.float32, tag="a0")
                    pa1 = pa1p.tile([128, HD], dt.float32, tag="a1")
                    pa.append((pa0, pa1))
                    pzc = pz_all[:, po_ + par * RK * H : po_ + (par + 1) * RK * H]
                    for jc in range(NCH):
                        w = msk_sb[:, jc * N + ic * 128 : jc * N + ic * 128 + 128]
                        st = jc == 0
                        sp = jc == NCH - 1
                        nc.tensor.matmul(
                            pa0[:, :], w,
                            v_sb[:, jc * RK * HD : jc * RK * HD + HD],
                            start=st, stop=sp,
                        )
                        nc.tensor.matmul(
                            pa1[:, :], w,
                            v_sb[:, jc * RK * HD + HD : jc * RK * HD + 2 * HD],
                            start=st, stop=sp,
                        )
                        nc.tensor.matmul(
                            pzc, w,
                            psicol_sb[:, jc * RK * H : (jc + 1) * RK * H],
                            start=st, stop=sp,
                        )
                # exits to bf16 (pairwise [128, 1024] working set)
                n0 = wk.tile([128, 2 * HD], dt.bfloat16, tag="n0")
                n1 = wk.tile([128, 2 * HD], dt.bfloat16, tag="n1")
                for par in range(2):
                    nc.scalar.activation(
                        n0[:, par * HD : (par + 1) * HD], pa[par][0][:, :], Act.Copy
                    )
                    nc.scalar.activation(
                        n1[:, par * HD : (par + 1) * HD], pa[par][1][:, :], Act.Copy
                    )
                # Z = pz[k0] + rho . pz[k1]  (fp32), both ics at once
                pzv = pz_all[:, po_ : po_ + 2 * RK * H].rearrange(
                    "p (i k h) -> p i k h", i=2, k=RK
                )
                rhob = rhof_sb[:, 2 * ip * H : (2 * ip + 2) * H]
                zt1 = wk.tile([128, 4 * H], dt.float32, tag="zt1")
                nc.vector.tensor_tensor(
                    out=zt1[:, 0 : 2 * H].rearrange("p (i h) -> p i h", i=2),
                    in0=pzv[:, :, 1, :],
                    in1=rhob.rearrange("p (i h) -> p i h", i=2),
                    op=Alu.mult,
                )
                nc.vector.tensor_tensor(
                    out=zt1[:, 2 * H : 4 * H].rearrange("p (i h) -> p i h", i=2),
                    in0=zt1[:, 0 : 2 * H].rearrange("p (i h) -> p i h", i=2),
                    in1=pzv[:, :, 0, :], op=Alu.add,
                )
                rz = wk.tile([128, 2 * H], dt.float32, tag="rz")
                nc.vector.reciprocal(
                    rz[:, :].rearrange("p (h s) -> p h s", s=1),
                    zt1[:, 2 * H : 4 * H].rearrange("p (h s) -> p h s", s=1),
                )
                # num = n0 + rhorep . n1 ; hh = num . rzrep
                num = wk.tile([128, 2 * HD], dt.bfloat16, tag="num")
                nc.vector.tensor_tensor(
                    out=num[:, :], in0=n1[:, :],
                    in1=rhorep_sb[:, 2 * ip * HD : (2 * ip + 2) * HD],
                    op=Alu.mult,
                )
                nc.vector.tensor_tensor(
                    out=num[:, :], in0=num[:, :], in1=n0[:, :], op=Alu.add
                )
                rzrep = wk.tile([128, 2 * HD], dt.bfloat16, tag="rzrep")
                nc.vector.tensor_copy(
                    out=rzrep[:, :].rearrange("p (h s) -> p h s", s=D),
                    in_=rz[:, :]
                    .rearrange("p (h s) -> p h s", s=1)
                    .to_broadcast([128, 2 * H, D]),
                )
                hh = wk.tile([128, 2 * HD], dt.bfloat16, tag="hh")
                nc.vector.tensor_tensor(
                    out=hh[:, :], in0=num[:, :], in1=rzrep[:, :], op=Alu.mult
                )
                # ELU(x) = max(x, min(exp(x)-1, 0))
                ee = wk.tile([128, 2 * HD], dt.bfloat16, tag="ee")
                nc.scalar.activation(ee[:, :], hh[:, :], Act.Exp)
                r1 = wk.tile([128, 2 * HD], dt.bfloat16, tag="r1")
                nc.vector.tensor_scalar(
                    out=r1[:, :], in0=ee[:, :], scalar1=-1.0, scalar2=0.0,
                    op0=Alu.add, op1=Alu.min,
                )
                nc.vector.tensor_tensor(
                    out=z_sb[:, 2 * ip * HD : (2 * ip + 2) * HD],
                    in0=hh[:, :], in1=r1[:, :], op=Alu.max,
                )

                # zT (8 transposed 128x128 blocks, kc-major then parity)
                pzi = pm2.tile([128, 8 * 128], dt.bfloat16, tag="mm2")
                for kc in range(4):
                    for par in range(2):
                        ic = 2 * ip + par
                        nc.tensor.transpose(
                            pzi[:, (kc * 2 + par) * 128 : (kc * 2 + par + 1) * 128],
                            z_sb[:, ic * HD + kc * 128 : ic * HD + kc * 128 + 128],
                            ident_b[:, :],
                        )
                nc.vector.tensor_copy(
                    out=zt_sb[:, :]
                    .rearrange("p (kc n) -> p kc n", n=N)[
                        :, :, 2 * ip * 128 : 2 * ip * 128 + 256
                    ],
                    in_=pzi[:, :].rearrange("p (kc s) -> p kc s", s=256),
                )

                # tl/tr via 2-col matmuls (u_l/u_r cols of wo), per ic
                for par in range(2):
                    ic = 2 * ip + par
                    ptl = pz_all[:, po_ + 32 + 2 * par : po_ + 34 + 2 * par]
                    for kc in range(4):
                        nc.tensor.matmul(
                            ptl,
                            zt_sb[:, kc * N + ic * 128 : kc * N + ic * 128 + 128],
                            wo_sb[:, kc * (C + 2) + C : kc * (C + 2) + C + 2],
                            start=(kc == 0), stop=False,
                        )
                    nc.tensor.matmul(
                        ptl, zt_one[:, ic * 128 : ic * 128 + 128],
                        wo_one[:, C : C + 2], start=False, stop=True,
                    )
                nc.vector.tensor_copy(
                    out=glgr[:, ip * 4 : ip * 4 + 4],
                    in_=pz_all[:, po_ + 32 : po_ + 36],
                )
                gb16 = wk.tile([128, 4], dt.bfloat16, tag="gb16")
                nc.vector.tensor_copy(
                    out=gb16[:, :], in_=glgr[:, ip * 4 : ip * 4 + 4]
                )
                pt2 = ptp.tile([4, 128], dt.bfloat16, tag="tp")
                nc.tensor.transpose(pt2[:, :], gb16[:, :], ident_b[:, :])
                gr2 = wk.tile([4, 128], dt.bfloat16, tag="gr2")
                nc.vector.tensor_copy(out=gr2[:, :], in_=pt2[:, :])
                for par in range(2):
                    ic = 2 * ip + par
                    nc.sync.dma_start(
                        out=rows_d[0:2, ic * 128 : (ic + 1) * 128],
                        in_=gr2[2 * par : 2 * par + 2, :],
                    )
                # tl row broadcast for this pair's segment lands during
                # phase 2, so the sweep prep can start before the last pair
                nc.sync.dma_start(
                    out=glb[:, 2 * ip * 128 : 2 * ip * 128 + 256],
                    in_=bcast128(rows_d[0:1, 2 * ip * 128 : 2 * ip * 128 + 256]),
                )

            # ---------- phase 3a: L2 sweep prep, segmented ----------
            # segment A = tl cols 0:768 (pairs 0-2, available while pair 3's
            # tail still runs); segment B = cols 768:1024 (pair 3)
            CA = 3  # chunks LeakyReLU'd by ACT (Prelu bias trick); rest DVE
            e2 = eb.tile([128, NCH * N], dt.bfloat16, tag="e")
            NB = NCH - CA
            t = tb.tile([128, NB * N], dt.bfloat16, tag="t")
            for s0, s1 in ((0, 768), (768, N)):
                sw = s1 - s0
                for c in range(CA):
                    nc.scalar.activation(
                        e2[:, c * N + s0 : c * N + s1], glb[:, s0:s1], Act.Prelu,
                        bias=glgr[:, c * 2 + 1 : c * 2 + 2], scale=1.0,
                        alpha=ALPHA,
                    )
                for c in range(CA, NCH):
                    nc.vector.tensor_scalar(
                        out=e2[:, c * N + s0 : c * N + s1], in0=glb[:, s0:s1],
                        scalar1=glgr[:, c * 2 + 1 : c * 2 + 2], scalar2=None,
                        op0=Alu.add,
                    )
                ev = e2[:, CA * N :].rearrange("p (c n) -> p c n", n=N)[
                    :, :, s0:s1
                ]
                tv = t[:, :].rearrange("p (c n) -> p c n", n=N)[:, :, s0:s1]
                nc.vector.tensor_scalar(
                    out=tv, in0=ev, scalar1=ALPHA, scalar2=None, op0=Alu.mult
                )
                nc.vector.tensor_tensor(out=ev, in0=tv, in1=ev, op=Alu.max)

            # ---------- phase 3b: g-projection (overlaps the sweep) ----------
            for ic in range(NCH):
                pg = pm2.tile([128, C + 2], dt.float32, tag="mm2")
                for kc in range(4):
                    nc.tensor.matmul(
                        pg[:, :],
                        zt_sb[:, kc * N + ic * 128 : kc * N + ic * 128 + 128],
                        wo_sb[:, kc * (C + 2) : (kc + 1) * (C + 2)],
                        start=(kc == 0), stop=False,
                    )
                nc.tensor.matmul(
                    pg[:, :], zt_one[:, ic * 128 : ic * 128 + 128], wo_one[:, :],
                    start=False, stop=True,
                )
                nc.vector.tensor_copy(
                    out=gx[:, ic * 260 : ic * 260 + C], in_=pg[:, 0:C]
                )

            # exp + mask in quarters for ACT/DVE pipelining
            QN = NCH * N // 4
            for q in range(4):
                nc.scalar.activation(
                    e2[:, q * QN : (q + 1) * QN], e2[:, q * QN : (q + 1) * QN],
                    Act.Exp,
                )
                nc.vector.tensor_tensor(
                    out=e2[:, q * QN : (q + 1) * QN],
                    in0=e2[:, q * QN : (q + 1) * QN],
                    in1=msk_sb[:, q * QN : (q + 1) * QN], op=Alu.mult,
                )

            # ---------- phase 4: L2 aggregation + ELU + residual ----------
            for ic in range(NCH):
                po = pa0p.tile([128, HD], dt.float32, tag="a0")
                for jc in range(NCH):
                    nc.tensor.matmul(
                        po[:, 0 : C + 1],
                        e2[:, jc * N + ic * 128 : jc * N + ic * 128 + 128],
                        gx[:, jc * 260 : jc * 260 + C + 1],
                        start=(jc == 0), stop=(jc == NCH - 1),
                    )
                rz2 = sm.tile([128, 1], dt.float32, tag="rz2")
                nc.vector.reciprocal(rz2[:, :], po[:, C : C + 1])
                y = sm.tile([128, C], dt.bfloat16, tag="y")
                nc.vector.tensor_scalar(
                    out=y[:, :], in0=po[:, 0:C], scalar1=rz2[:, :], scalar2=None,
                    op0=Alu.mult,
                )
                e3 = sm.tile([128, C], dt.bfloat16, tag="e3")
                nc.scalar.activation(e3[:, :], y[:, :], Act.Exp)
                r2 = sm.tile([128, C], dt.bfloat16, tag="r2")
                nc.vector.tensor_scalar(
                    out=r2[:, :], in0=e3[:, :], scalar1=-1.0, scalar2=0.0,
                    op0=Alu.add, op1=Alu.min,
                )
                el = sm.tile([128, C], dt.bfloat16, tag="el")
                nc.vector.tensor_tensor(
                    out=el[:, :], in0=y[:, :], in1=r2[:, :], op=Alu.max
                )
                xs5 = sm.tile([128, F], dt.float32, tag="xs5")
                nc.sync.dma_start(
                    out=xs5[:, :], in_=xs[ic * 128 : (ic + 1) * 128, :]
                )
                ofin = sm.tile([128, C], dt.float32, tag="ofin")
                nc.vector.tensor_tensor(
                    out=ofin[:, :], in0=el[:, :], in1=xs5[:, :], op=Alu.add,
                )
                nc.sync.dma_start(
                    out=out_d[ic * 128 : (ic + 1) * 128, :], in_=ofin[:, :]
                )

    nc.compile()
    return nc


def get_program():
    if "nc" not in _CACHE:
        _CACHE["nc"] = _build_program()
    return _CACHE["nc"]


def _fit_rank2(sl, sr, ngrid=257):
    """Fit g(x+y)=exp(LeakyReLU(x+y)) ~= sum_k phi_k(x) psi_k(y), rank RK,
    on the realized box. Returns (rho[N] fp32, psi[N, RK] fp32)."""
    pad_x = 1e-3 * (sl.max() - sl.min()) + 1e-6
    pad_y = 1e-3 * (sr.max() - sr.min()) + 1e-6
    xs = np.linspace(sl.min() - pad_x, sl.max() + pad_x, ngrid)
    ys = np.linspace(sr.min() - pad_y, sr.max() + pad_y, ngrid)
    ss = xs[:, None] + ys[None, :]
    G = np.exp(np.where(ss >= 0, ss, ALPHA * ss))
    U, S, Vt = np.linalg.svd(G, full_matrices=False)
    phi_g = U[:, :RK] * S[:RK]
    psi_g = Vt[:RK].T
    if phi_g[:, 0].mean() < 0:
        phi_g[:, 0] *= -1.0
        psi_g[:, 0] *= -1.0
    phi = np.stack([np.interp(sl, xs, phi_g[:, k]) for k in range(RK)], axis=1)
    psi = np.stack([np.interp(sr, ys, psi_g[:, k]) for k in range(RK)], axis=1)
    assert np.all(phi[:, 0] > 0), "phi0 must be positive"
    rho = phi[:, 1] / phi[:, 0]
    return rho.astype(np.float32), psi.astype(np.float32)


def make_in_maps(x, adj, W, Wb, a, ab, Wo, Wob, ao, aob):
    x = np.asarray(x, np.float32)
    adj = np.asarray(adj)
    W = np.asarray(W, np.float32)
    Wb = np.asarray(Wb, np.float32)
    a = np.asarray(a, np.float32)
    ab = np.asarray(ab, np.float32)
    Wo = np.asarray(Wo, np.float32)
    Wob = np.asarray(Wob, np.float32)
    ao = np.asarray(ao, np.float32)
    aob = np.asarray(aob, np.float32)

    # W_all[f, h*D+d] = W[h, f, d];  Wb row flattened the same way
    W_all = W.transpose(1, 0, 2).reshape(F, HD)
    wb_row = Wb.reshape(1, HD)
    wp = np.concatenate([W_all, wb_row], axis=0).astype(BF16)  # [257, 512]

    # sl/sr per-node linear maps of x, folded on the host (fp32)
    V_l = np.einsum("hfd,hd->fh", W, a[:, :D]).astype(np.float32)
    V_r = np.einsum("hfd,hd->fh", W, a[:, D:]).astype(np.float32)
    const_l = (Wb * a[:, :D]).sum(1) + ab  # [H]
    const_r = (Wb * a[:, D:]).sum(1)
    sl_all = np.einsum("bnf,fh->bhn", x, V_l) + const_l[None, :, None]  # [B,H,N]
    sr_all = np.einsum("bnf,fh->bhn", x, V_r) + const_r[None, :, None]  # [B,H,N]

    u_l = Wo @ ao[:C]  # [512]
    u_r = Wo @ ao[C:]
    wo_top = np.concatenate([Wo, u_l[:, None], u_r[:, None]], axis=1)  # [512, 258]
    wo_bot = np.concatenate(
        [Wob, [Wob @ ao[:C] + aob], [Wob @ ao[C:]]]
    )[None, :]  # [1, 258]
    wo_ext = np.concatenate([wo_top, wo_bot], axis=0).astype(BF16)  # [513, 258]

    ones_row = np.ones((1, N), BF16)
    in_maps = []
    for b in range(B):
        psicol = np.empty((N, RK * H), np.float32)
        rhof = np.empty((N, H), np.float32)
        for hh in range(H):
            rho, psi = _fit_rank2(sl_all[b, hh], sr_all[b, hh])
            rhof[:, hh] = rho
            for k in range(RK):
                psicol[:, k * H + hh] = psi[:, k]
        psicol_b = psicol.astype(BF16)
        psirep_b = np.repeat(
            psicol_b.reshape(N, RK * H), D, axis=1
        )  # [N, RK*H*D], bf16 values replicated per head block
        rhorep_b = np.repeat(rhof.astype(BF16), D, axis=1)  # [N, HD]
        xtb = np.concatenate([x[b].T.astype(BF16), ones_row], axis=0)  # [257, 1024]
        mb = np.where(adj[b].T > 0, np.float32(1.0), np.float32(0.0)).astype(BF16)
        in_maps.append(
            {
                "xt": np.ascontiguousarray(xtb),
                "xs": np.ascontiguousarray(x[b]),
                "msk": np.ascontiguousarray(mb),
                "wp": wp,
                "psicol": psicol_b,
                "psirep": np.ascontiguousarray(psirep_b),
                "rhorep": np.ascontiguousarray(rhorep_b),
                "rhof": rhof,
                "wo": wo_ext,
            }
        )
    return in_maps


def kernel(**inputs) -> np.ndarray:
    from concourse.bass_utils import run_bass_kernel_spmd

    nc = get_program()
    in_maps = make_in_maps(**inputs)
    res = run_bass_kernel_spmd(nc, in_maps, core_ids=list(range(B)))
    return np.stack([res.results[b]["out"] for b in range(B)], axis=0)
